# revision 6
# baseline (speedup 1.0000x reference)
"""GwcVolume (group-wise correlation cost volume) Trainium2 kernel, v2.

cost[b,g,d,h,x] = mean_c( lf[b, g*8+c, h, x] * rf[b, g*8+c, h, x-d] ),
zero for x < d.  Shapes: lf/rf [2, 320, 128, 240] f32 -> out [2, 40, 48, 128, 240] f32.

Sharding: h-axis split across 8 cores (16 rows each). Correlation is along w
only, so shards are fully independent and each core reads just its h-band.

v2 design (vs v1 baseline at 692 us):
  - Output stored fp16 in HBM (host upcasts): halves write traffic.
  - x>=d trimming: products/matmuls/drains only cover x in [d, W). The x<d
    zeros come from a per-(b,d) memset of the staging tile's prefix.
    Odd-d slices stay 4B-aligned (DVE 2x mode) via +1-shifted lf copies.
  - Group-sum matmuls use three zero-padded [*, 128] fp16 stationaries
    (S0: cols 0:16, S1: cols 16:32, S2: cols 32:40) accumulated into ONE
    psum region per j-pair: all 40 groups land on contiguous psum rows
    0:40, so each (b,hg,d) drains with a single ScalarE copy and each
    (b,d) writes with large 7680B/partition DMA blocks. Full 128-wide
    stationaries also enable the compiler's fast-weight-load path.
  - chunk2 (channels 256:320, 64-wide) is packed 2-h-halves-per-partition
    so its elementwise products use all 128 DVE/GPSIMD lanes; its matmuls
    use tile_position (0,0)/(64,0) to contract partition halves.
  - chunk2 products run on GPSIMD to offload the DVE bottleneck.
"""

import numpy as np

import concourse.bass as bass
import concourse.tile as tile
from concourse import mybir
from concourse.bass_utils import run_bass_kernel_spmd

B = 2
C = 320
H = 128
W = 240
G = 40
CPG = 8
D = 48
NCORES = 8
HS = H // NCORES  # h rows per core (16)
F16 = mybir.dt.float16
F32 = mybir.dt.float32


def split_multi_waits(nc, limit=1):
    """Walrus in this container rejects instructions carrying more than
    `limit` semaphore waits. Move excess waits onto preceding NoOps on the
    same engine (waits execute before the instruction, in stream order)."""
    n_split = 0
    for fn in nc.m.functions:
        for bb in fn.blocks:
            insts = bb.instructions
            i = 0
            while i < len(insts):
                inst = insts[i]
                si = inst.sync_info
                if si is not None and len(si.on_wait) > limit:
                    waits = list(si.on_wait)
                    keep = waits[-limit:]
                    extra = waits[:-limit]
                    new_insts = []
                    for j in range(0, len(extra), limit):
                        chunk = extra[j : j + limit]
                        nop = mybir.InstNoOp(
                            name=nc.get_next_instruction_name(),
                            engine=inst.engine,
                            ins=[],
                            outs=[],
                            sync_info=mybir.SyncInfo(on_wait=chunk, on_update=[]),
                        )
                        new_insts.append(nop)
                    inst.sync_info = mybir.SyncInfo(
                        on_wait=keep, on_update=list(si.on_update)
                    )
                    insts[i:i] = new_insts
                    i += len(new_insts)
                    n_split += 1
                i += 1
    return n_split


def build_bass(n_b=B, n_d=D):
    nc = bass.Bass("TRN2", target_bir_lowering=False, debug=False, num_devices=NCORES)
    lf = nc.dram_tensor("lf", [B, C, HS, W], F32, kind="ExternalInput").ap()
    rf = nc.dram_tensor("rf", [B, C, HS, W], F32, kind="ExternalInput").ap()
    s0 = nc.dram_tensor("s0", [128, 128], F16, kind="ExternalInput").ap()
    s1 = nc.dram_tensor("s1", [128, 128], F16, kind="ExternalInput").ap()
    s2 = nc.dram_tensor("s2", [64, 128], F16, kind="ExternalInput").ap()
    outp = nc.dram_tensor("outp", [B, G, D, HS, W], F16, kind="ExternalOutput").ap()

    with tile.TileContext(nc) as tc:
        with (
            tc.tile_pool(name="const", bufs=1) as cpool,
            tc.tile_pool(name="ins", bufs=2) as ipool,
            tc.tile_pool(name="prod", bufs=3) as ppool,
            tc.tile_pool(name="prod2", bufs=3) as p2pool,
            tc.tile_pool(name="outs", bufs=4) as opool,
            tc.tile_pool(name="psum", bufs=2, space="PSUM") as qpool,
        ):
            s0_t = cpool.tile([128, 128], F16)
            nc.sync.dma_start(s0_t[:], s0[:])
            s1_t = cpool.tile([128, 128], F16)
            nc.sync.dma_start(s1_t[:], s1[:])
            s2_t = cpool.tile([64, 128], F16)
            nc.sync.dma_start(s2_t[:], s2[:])
            # Walrus requires ifmap and weights to start at the same SBUF
            # partition: keep a second s2 copy on partitions 64:128 for the
            # matmuls whose moving data lives in the upper half.
            s2b_t = cpool.tile([128, 128], F16)
            nc.sync.dma_start(s2b_t[64:128, :], s2[:])

            for b in range(n_b):
                # Stage this batch's full 16-row h-band: chunk0/1 channels
                # interleaved on free dim, chunk2 h-half-packed on partitions.
                ins = {}
                for hg in range(2):
                    h0 = hg * 8
                    l01 = ipool.tile([128, 2, 8, W], F16, tag=f"l01_{hg}")
                    l01o = ipool.tile([128, 2, 8, W], F16, tag=f"l01o_{hg}")
                    r01 = ipool.tile([128, 2, 8, W], F16, tag=f"r01_{hg}")
                    for cc in range(2):
                        c0 = 128 * cc
                        nc.gpsimd.dma_start(
                            l01[:, cc, :, :], lf[b, c0 : c0 + 128, h0 : h0 + 8, :]
                        )
                        nc.gpsimd.dma_start(
                            r01[:, cc, :, :], rf[b, c0 : c0 + 128, h0 : h0 + 8, :]
                        )
                        nc.gpsimd.dma_start(
                            l01o[:, cc, :, 0 : W - 1],
                            lf[b, c0 : c0 + 128, h0 : h0 + 8, 1:W],
                        )
                    l2 = ipool.tile([128, 4, W], F16, tag=f"l2_{hg}")
                    l2o = ipool.tile([128, 4, W], F16, tag=f"l2o_{hg}")
                    r2 = ipool.tile([128, 4, W], F16, tag=f"r2_{hg}")
                    for hh in range(2):
                        p0 = 64 * hh
                        ha = h0 + 4 * hh
                        nc.gpsimd.dma_start(
                            l2[p0 : p0 + 64, :, :], lf[b, 256:320, ha : ha + 4, :]
                        )
                        nc.gpsimd.dma_start(
                            r2[p0 : p0 + 64, :, :], rf[b, 256:320, ha : ha + 4, :]
                        )
                        nc.gpsimd.dma_start(
                            l2o[p0 : p0 + 64, :, 0 : W - 1],
                            lf[b, 256:320, ha : ha + 4, 1:W],
                        )
                    ins[hg] = (l01, l01o, r01, l2, l2o, r2)

                for d in range(n_d):
                    wd = W - d
                    ot = opool.tile([40, 16, W], F16, tag="ot")
                    for hg in range(2):
                        l01, l01o, r01, l2, l2o, r2 = ins[hg]
                        pt = ppool.tile([128, 2, 8, W], F16, tag="pt")
                        p2 = p2pool.tile([128, 4, W], F16, tag="p2")
                        if d % 2 == 0:
                            lsl = l01[:, :, :, d : d + wd]
                            ls2 = l2[:, :, d : d + wd]
                        else:
                            lsl = l01o[:, :, :, d - 1 : d - 1 + wd]
                            ls2 = l2o[:, :, d - 1 : d - 1 + wd]
                        nc.vector.tensor_mul(pt[:, :, :, 0:wd], lsl, r01[:, :, :, 0:wd])
                        nc.gpsimd.tensor_mul(p2[:, :, 0:wd], ls2, r2[:, :, 0:wd])

                        ps = qpool.tile([128, 8, 256], F32)
                        for j in range(4):
                            nc.tensor.matmul(
                                ps[:, 2 * j : 2 * j + 2, 0:wd],
                                s0_t[:, :],
                                pt[:, 0, 2 * j : 2 * j + 2, 0:wd],
                                start=True,
                                stop=False,
                                skip_group_check=True,
                            )
                        for j in range(4):
                            nc.tensor.matmul(
                                ps[:, 2 * j : 2 * j + 2, 0:wd],
                                s1_t[:, :],
                                pt[:, 1, 2 * j : 2 * j + 2, 0:wd],
                                start=False,
                                stop=False,
                                skip_group_check=True,
                            )
                        for j in range(4):
                            hh = j // 2
                            p0 = 64 * hh
                            hsub = 2 * (j % 2)
                            s2w = s2_t[:, :] if hh == 0 else s2b_t[64:128, :]
                            nc.tensor.matmul(
                                ps[:, 2 * j : 2 * j + 2, 0:wd],
                                s2w,
                                p2[p0 : p0 + 64, hsub : hsub + 2, 0:wd],
                                start=False,
                                stop=True,
                                skip_group_check=True,
                                tile_position=(p0, 0),
                            )
                        nc.scalar.copy(
                            ot[0:40, 8 * hg : 8 * hg + 8, d:W], ps[0:40, 0:8, 0:wd]
                        )
                    if d > 0:
                        nc.gpsimd.memset(ot[:, :, 0:d], 0.0)
                    nc.sync.dma_start(outp[b, 0:20, d, :, :], ot[0:20, :, :])
                    nc.sync.dma_start(outp[b, 20:40, d, :, :], ot[20:40, :, :])
    split_multi_waits(nc)
    return nc


def make_smats():
    s0 = np.zeros((128, 128), np.float16)
    s1 = np.zeros((128, 128), np.float16)
    s2 = np.zeros((64, 128), np.float16)
    for g in range(16):
        s0[g * CPG : (g + 1) * CPG, g] = 1.0 / CPG
        s1[g * CPG : (g + 1) * CPG, 16 + g] = 1.0 / CPG
    for g in range(8):
        s2[g * CPG : (g + 1) * CPG, 32 + g] = 1.0 / CPG
    return s0, s1, s2


_NC_CACHE = {}


def _get_nc(key=(B, D)):
    if key not in _NC_CACHE:
        _NC_CACHE[key] = build_bass(*key)
    return _NC_CACHE[key]


def run_sharded(lf, rf, nc=None, trace=False, tmpdir=None):
    """lf/rf: full [2, 320, 128, 240] f32 numpy arrays. Returns (out, results)."""
    if nc is None:
        nc = _get_nc()
    s0, s1, s2 = make_smats()
    in_maps = []
    for k in range(NCORES):
        in_maps.append(
            {
                "lf": np.ascontiguousarray(lf[:, :, k * HS : (k + 1) * HS, :]),
                "rf": np.ascontiguousarray(rf[:, :, k * HS : (k + 1) * HS, :]),
                "s0": s0,
                "s1": s1,
                "s2": s2,
            }
        )
    res = run_bass_kernel_spmd(
        nc, in_maps, list(range(NCORES)), trace=trace, tmpdir=tmpdir
    )
    out = np.empty((B, G, D, H, W), np.float32)
    for k in range(NCORES):
        out[:, :, :, k * HS : (k + 1) * HS, :] = res.results[k]["outp"].astype(
            np.float32
        )
    return out, res


def kernel(**inputs):
    lf = np.asarray(inputs["left_feature"], dtype=np.float32)
    rf = np.asarray(inputs["right_feature"], dtype=np.float32)
    out, _ = run_sharded(lf, rf)
    return out


if __name__ == "__main__":
    rng = np.random.default_rng(0)
    lf = rng.standard_normal((B, C, H, W), dtype=np.float32)
    rf = rng.standard_normal((B, C, H, W), dtype=np.float32)
    out, _ = run_sharded(lf, rf)
    print(out.shape, out.dtype, float(np.abs(out).max()))


# revision 9
# speedup vs baseline: 1.1516x; 1.1516x over previous
"""GwcVolume (group-wise correlation cost volume) Trainium2 kernel, v2.

cost[b,g,d,h,x] = mean_c( lf[b, g*8+c, h, x] * rf[b, g*8+c, h, x-d] ),
zero for x < d.  Shapes: lf/rf [2, 320, 128, 240] f32 -> out [2, 40, 48, 128, 240] f32.

Sharding: h-axis split across 8 cores (16 rows each). Correlation is along w
only, so shards are fully independent and each core reads just its h-band.

v2 design (vs v1 baseline at 692 us):
  - Output stored fp16 in HBM (host upcasts): halves write traffic.
  - x>=d trimming: products/matmuls/drains only cover x in [d, W). The x<d
    zeros come from a per-(b,d) memset of the staging tile's prefix.
    Odd-d slices stay 4B-aligned (DVE 2x mode) via +1-shifted lf copies.
  - Group-sum matmuls use three zero-padded [*, 128] fp16 stationaries
    (S0: cols 0:16, S1: cols 16:32, S2: cols 32:40) accumulated into ONE
    psum region per j-pair: all 40 groups land on contiguous psum rows
    0:40, so each (b,hg,d) drains with a single ScalarE copy and each
    (b,d) writes with large 7680B/partition DMA blocks. Full 128-wide
    stationaries also enable the compiler's fast-weight-load path.
  - chunk2 (channels 256:320, 64-wide) is packed 2-h-halves-per-partition
    so its elementwise products use all 128 DVE/GPSIMD lanes; its matmuls
    use tile_position (0,0)/(64,0) to contract partition halves.
  - chunk2 products run on GPSIMD to offload the DVE bottleneck.
"""

import numpy as np

import concourse.bass as bass
import concourse.tile as tile
from concourse import mybir
from concourse.bass_utils import run_bass_kernel_spmd

B = 2
C = 320
H = 128
W = 240
G = 40
CPG = 8
D = 48
NCORES = 8
HS = H // NCORES  # h rows per core (16)
F16 = mybir.dt.float16
F32 = mybir.dt.float32


def split_multi_waits(nc, limit=1):
    """Walrus in this container rejects instructions carrying more than
    `limit` semaphore waits. Move excess waits onto preceding NoOps on the
    same engine (waits execute before the instruction, in stream order)."""
    n_split = 0
    for fn in nc.m.functions:
        for bb in fn.blocks:
            insts = bb.instructions
            i = 0
            while i < len(insts):
                inst = insts[i]
                si = inst.sync_info
                if si is not None and len(si.on_wait) > limit:
                    waits = list(si.on_wait)
                    keep = waits[-limit:]
                    extra = waits[:-limit]
                    new_insts = []
                    for j in range(0, len(extra), limit):
                        chunk = extra[j : j + limit]
                        nop = mybir.InstNoOp(
                            name=nc.get_next_instruction_name(),
                            engine=inst.engine,
                            ins=[],
                            outs=[],
                            sync_info=mybir.SyncInfo(on_wait=chunk, on_update=[]),
                        )
                        new_insts.append(nop)
                    inst.sync_info = mybir.SyncInfo(
                        on_wait=keep, on_update=list(si.on_update)
                    )
                    insts[i:i] = new_insts
                    i += len(new_insts)
                    n_split += 1
                i += 1
    return n_split


def build_bass(n_b=B, n_d=D):
    nc = bass.Bass("TRN2", target_bir_lowering=False, debug=False, num_devices=NCORES)
    lf = nc.dram_tensor("lf", [B, C, HS, W], F32, kind="ExternalInput").ap()
    rf = nc.dram_tensor("rf", [B, C, HS, W], F32, kind="ExternalInput").ap()
    s0 = nc.dram_tensor("s0", [128, 128], F16, kind="ExternalInput").ap()
    s1 = nc.dram_tensor("s1", [128, 128], F16, kind="ExternalInput").ap()
    s2 = nc.dram_tensor("s2", [64, 128], F16, kind="ExternalInput").ap()
    outp = nc.dram_tensor("outp", [B, G, D, HS, W], F16, kind="ExternalOutput").ap()

    with tile.TileContext(nc) as tc:
        with (
            tc.tile_pool(name="const", bufs=1) as cpool,
            tc.tile_pool(name="ins", bufs=2) as ipool,
            tc.tile_pool(name="prod", bufs=3) as ppool,
            tc.tile_pool(name="prod2", bufs=3) as p2pool,
            tc.tile_pool(name="outs", bufs=4) as opool,
            tc.tile_pool(name="psum", bufs=2, space="PSUM") as qpool,
        ):
            s0_t = cpool.tile([128, 128], F16)
            nc.sync.dma_start(s0_t[:], s0[:])
            s1_t = cpool.tile([128, 128], F16)
            nc.sync.dma_start(s1_t[:], s1[:])
            s2_t = cpool.tile([64, 128], F16)
            nc.sync.dma_start(s2_t[:], s2[:])
            # Walrus requires ifmap and weights to start at the same SBUF
            # partition: keep a second s2 copy on partitions 64:128 for the
            # matmuls whose moving data lives in the upper half.
            s2b_t = cpool.tile([128, 128], F16)
            nc.sync.dma_start(s2b_t[64:128, :], s2[:])

            for b in range(n_b):
                # Stage this batch's full 16-row h-band: chunk0/1 channels
                # interleaved on free dim, chunk2 h-half-packed on partitions.
                ins = {}
                for hg in range(2):
                    h0 = hg * 8
                    l01 = ipool.tile([128, 2, 8, W], F16, tag=f"l01_{hg}")
                    r01 = ipool.tile([128, 2, 8, W], F16, tag=f"r01_{hg}")
                    # r01o[..., 1+x] = rf[..., x]; col 0 is junk. Odd-d slices
                    # then start at even offsets with even widths so the DVE
                    # 2x perf mode stays eligible for every d.
                    r01o = ipool.tile([128, 2, 8, W + 2], F16, tag=f"r01o_{hg}")
                    for cc in range(2):
                        c0 = 128 * cc
                        nc.gpsimd.dma_start(
                            l01[:, cc, :, :], lf[b, c0 : c0 + 128, h0 : h0 + 8, :]
                        )
                        nc.gpsimd.dma_start(
                            r01[:, cc, :, :], rf[b, c0 : c0 + 128, h0 : h0 + 8, :]
                        )
                        nc.scalar.copy(r01o[:, cc, :, 1 : W + 1], r01[:, cc, :, :])
                    l2 = ipool.tile([128, 4, W], F16, tag=f"l2_{hg}")
                    r2 = ipool.tile([128, 4, W], F16, tag=f"r2_{hg}")
                    r2o = ipool.tile([128, 4, W + 2], F16, tag=f"r2o_{hg}")
                    for hh in range(2):
                        p0 = 64 * hh
                        ha = h0 + 4 * hh
                        nc.gpsimd.dma_start(
                            l2[p0 : p0 + 64, :, :], lf[b, 256:320, ha : ha + 4, :]
                        )
                        nc.gpsimd.dma_start(
                            r2[p0 : p0 + 64, :, :], rf[b, 256:320, ha : ha + 4, :]
                        )
                    nc.scalar.copy(r2o[:, :, 1 : W + 1], r2[:, :, :])
                    ins[hg] = (l01, r01, r01o, l2, r2, r2o)

                for d in range(n_d):
                    wd = W - d
                    # Even d: products at columns [0:wd] = x in [d, W).
                    # Odd d: compute one extra (junk) leading column so every
                    # slice keeps an even offset and even width: products at
                    # columns [0:wd+1], of which [1:wd+1] are x in [d, W).
                    if d % 2 == 0:
                        lo, wc, mo = d, wd, 0
                    else:
                        lo, wc, mo = d - 1, wd + 1, 1
                    ot = opool.tile([40, 16, W], F16, tag="ot")
                    for hg in range(2):
                        l01, r01, r01o, l2, r2, r2o = ins[hg]
                        pt = ppool.tile([128, 2, 8, W + 2], F16, tag="pt")
                        p2 = p2pool.tile([128, 4, W + 2], F16, tag="p2")
                        for cc in range(2):
                            if d % 2 == 0:
                                rsl = r01[:, cc, :, 0:wc]
                            else:
                                rsl = r01o[:, cc, :, 0:wc]
                            nc.vector.tensor_mul(
                                pt[:, cc, :, 0:wc], l01[:, cc, :, lo : lo + wc], rsl
                            )
                        r2sel = r2 if d % 2 == 0 else r2o
                        nc.gpsimd.tensor_mul(
                            p2[:, 0:2, 0:wc],
                            l2[:, 0:2, lo : lo + wc],
                            r2sel[:, 0:2, 0:wc],
                        )
                        nc.vector.tensor_mul(
                            p2[:, 2:4, 0:wc],
                            l2[:, 2:4, lo : lo + wc],
                            r2sel[:, 2:4, 0:wc],
                        )

                        ps = qpool.tile([128, 8, 256], F32)
                        for j in range(4):
                            nc.tensor.matmul(
                                ps[:, 2 * j : 2 * j + 2, 0:wd],
                                s0_t[:, :],
                                pt[:, 0, 2 * j : 2 * j + 2, mo : mo + wd],
                                start=True,
                                stop=False,
                                skip_group_check=True,
                            )
                        for j in range(4):
                            nc.tensor.matmul(
                                ps[:, 2 * j : 2 * j + 2, 0:wd],
                                s1_t[:, :],
                                pt[:, 1, 2 * j : 2 * j + 2, mo : mo + wd],
                                start=False,
                                stop=False,
                                skip_group_check=True,
                            )
                        for j in range(4):
                            hh = j // 2
                            p0 = 64 * hh
                            hsub = 2 * (j % 2)
                            s2w = s2_t[:, :] if hh == 0 else s2b_t[64:128, :]
                            nc.tensor.matmul(
                                ps[:, 2 * j : 2 * j + 2, 0:wd],
                                s2w,
                                p2[p0 : p0 + 64, hsub : hsub + 2, mo : mo + wd],
                                start=False,
                                stop=True,
                                skip_group_check=True,
                                tile_position=(p0, 0),
                            )
                        nc.scalar.copy(
                            ot[0:40, 8 * hg : 8 * hg + 8, d:W], ps[0:40, 0:8, 0:wd]
                        )
                    if d > 0:
                        nc.gpsimd.memset(ot[:, :, 0:d], 0.0)
                    nc.sync.dma_start(outp[b, 0:20, d, :, :], ot[0:20, :, :])
                    nc.sync.dma_start(outp[b, 20:40, d, :, :], ot[20:40, :, :])
    split_multi_waits(nc)
    return nc


def make_smats():
    s0 = np.zeros((128, 128), np.float16)
    s1 = np.zeros((128, 128), np.float16)
    s2 = np.zeros((64, 128), np.float16)
    for g in range(16):
        s0[g * CPG : (g + 1) * CPG, g] = 1.0 / CPG
        s1[g * CPG : (g + 1) * CPG, 16 + g] = 1.0 / CPG
    for g in range(8):
        s2[g * CPG : (g + 1) * CPG, 32 + g] = 1.0 / CPG
    return s0, s1, s2


_NC_CACHE = {}


def _get_nc(key=(B, D)):
    if key not in _NC_CACHE:
        _NC_CACHE[key] = build_bass(*key)
    return _NC_CACHE[key]


def run_sharded(lf, rf, nc=None, trace=False, tmpdir=None):
    """lf/rf: full [2, 320, 128, 240] f32 numpy arrays. Returns (out, results)."""
    if nc is None:
        nc = _get_nc()
    s0, s1, s2 = make_smats()
    in_maps = []
    for k in range(NCORES):
        in_maps.append(
            {
                "lf": np.ascontiguousarray(lf[:, :, k * HS : (k + 1) * HS, :]),
                "rf": np.ascontiguousarray(rf[:, :, k * HS : (k + 1) * HS, :]),
                "s0": s0,
                "s1": s1,
                "s2": s2,
            }
        )
    res = run_bass_kernel_spmd(
        nc, in_maps, list(range(NCORES)), trace=trace, tmpdir=tmpdir
    )
    out = np.empty((B, G, D, H, W), np.float32)
    for k in range(NCORES):
        out[:, :, :, k * HS : (k + 1) * HS, :] = res.results[k]["outp"].astype(
            np.float32
        )
    return out, res


def kernel(**inputs):
    lf = np.asarray(inputs["left_feature"], dtype=np.float32)
    rf = np.asarray(inputs["right_feature"], dtype=np.float32)
    out, _ = run_sharded(lf, rf)
    return out


if __name__ == "__main__":
    rng = np.random.default_rng(0)
    lf = rng.standard_normal((B, C, H, W), dtype=np.float32)
    rf = rng.standard_normal((B, C, H, W), dtype=np.float32)
    out, _ = run_sharded(lf, rf)
    print(out.shape, out.dtype, float(np.abs(out).max()))
